# revision 32
# baseline (speedup 1.0000x reference)
"""MoE layer (top-2 of 24 experts, d_model=1024, d_ff=4096, T=4096 tokens)
on 8 Trainium2 NeuronCores.

Strategy (expert-parallel, host-routed):
  - Host computes the gate, top-2 ids and softmax probs, gathers each
    expert's tokens into a transposed, k-packed buffer per slot.
  - Experts sharded 3 per core, balanced by token count (sorted-deal into
    3 slots); slot capacity = max count in that octile, 8-aligned exact.
  - Per expert on device (all matmul free dims = exact token count C):
      phase A: ht[m] = gelu(w1_km.T @ xT + b1)    32 m-groups, N=C
      phase B (transposed): 8 PSUM-resident banks pb[md] = yT d-chunks,
               k-outer sweep so w2 streams HBM exactly once:
               pb[md] += w2_k[:, md].T @ ht[k]    N=C, no 128-padding
      drain:   last 4 k-steps run md-outer; each bank is copied (DVE) into
               a packed [128, 8*C] out tile and shipped as two wide DMAs
               (prob scaling happens on host).
  - Weights stream in fine chunks (w1: 256-col, w2: 2 k-tiles) through
    deep tile pools (18 bufs each) so the two HWDGE rings (sync: w1+y,
    scalar: w2+xt) never head-of-line block at phase boundaries; DMAs are
    emitted in deadline-sorted order against a 2.366 GHz PE clock model.
  - ~14 warmup matmuls on a zeroed scratch tile keep the PE busy (and the
    HAM clock un-throttled) through the startup DMA window.
  - Host scatters the two per-token expert outputs back together.

Matmuls in bf16 with fp32 PSUM accumulation (rel err ~4e-3); b1 applied
exactly as the ACT per-partition bias.
"""

import numpy as np
import ml_dtypes

P = 128
D_MODEL = 1024
D_FF = 4096
NUM_EXPERTS = 24
TOP_K = 2
N_CORES = 8
E_LOC = NUM_EXPERTS // N_CORES   # 3 experts per core
KD = D_MODEL // P                # 8  k-chunks over d_model
KF = D_FF // P                   # 32 k-chunks over d_ff
MD = D_MODEL // P                # 8  output d-chunks (phase B)
G = 4                            # w2 k-tiles packed per DMA chunk
BF16 = ml_dtypes.bfloat16
NWARM = 16                       # PE warmup matmuls
CPUS = 2366.0                    # measured PE cycles per us


def _w1_chunks(e):
    """(col_start, width) chunks of w1's 4096 columns for expert slot e.
    First-processed expert leads with two 256-col chunks for a fast start.
    Chunks are kept >= 512 cols (1 MB) elsewhere: the Tile framework
    round-robins all DMAs over 8 completion semaphores, so many small
    transfers serialize the triggers across both HWDGE queues."""
    if e == 0:
        return [(0, 128), (128, 128), (256, 256)] + \
            [(512 * i, 512) for i in range(1, 8)]
    return [(512 * i, 512) for i in range(8)]


def _build(Cs, repeat=1):
    """Per-core Bass program (SPMD: same program, per-core data).

    Cs: per-slot token capacities (8-aligned, each <= 512).
    """
    import concourse.bacc as bacc
    import concourse.mybir as mybir
    from concourse.tile import TileContext

    dt = mybir.dt.bfloat16
    f32 = mybir.dt.float32
    C0 = Cs[0]

    # model timeline (us) for DMA deadline sorting
    tA = [256.0 * C / CPUS for C in Cs]
    t0 = []
    t = 0.0
    for j in range(E_LOC):
        t0.append(t)
        t += 2.0 * tA[j]

    # (deadline, ring, kind, e, idx)
    # ring 0=sync HWDGE (w1, xt, y), 1=scalar HWDGE (startup only — any
    # blocked trigger in the scalar stream head-of-line blocks the gelus
    # behind it), 2=gpsimd SWDGE (w2: own queue + own 8-sem pool, fully
    # decoupled from the HWDGE semaphore rotation)
    events = []
    for e in range(E_LOC):
        for ci, (cs, w) in enumerate(_w1_chunks(e)):
            if e == 0 and ci in (0, 1):
                events.append((-99.0 + 0.5 * ci, 1, 'w1', e, ci))
            elif e == 0:
                # expert 0's phase A window is supply-tight (cold DMA
                # window): offload three late chunks to the scalar queue
                ring = 1 if ci in (5, 6, 8) else 0
                events.append((tA[0] * cs / D_FF - 14.0, ring, 'w1', e, ci))
            else:
                events.append((t0[e] + tA[e] * cs / D_FF - 14.0, 0,
                               'w1', e, ci))
        for kc in range(KF // G):
            events.append((t0[e] + tA[e] * (1.0 + kc * G / KF) - 20.0,
                           2, 'w2', e, kc))
        if e > 0:
            events.append((t0[e] - 28.0, 2, 'xt', e, 0))
    events.append((-98.0, 1, 'b1', 0, 0))
    events.append((-100.0, 0, 'xt', 0, 0))
    events.sort(key=lambda ev: (ev[0], ev[1]))

    nc = bacc.Bacc(None, target_bir_lowering=False)
    xt_d = [nc.dram_tensor(f"xt{j}", [P, KD * Cs[j]], dt, kind="ExternalInput")
            for j in range(E_LOC)]
    w1 = nc.dram_tensor("w1", [E_LOC, P, KD * D_FF], dt, kind="ExternalInput")
    w2 = nc.dram_tensor("w2", [E_LOC, KF // G, P, G * D_MODEL], dt,
                        kind="ExternalInput")
    # b1 padded to 2 KB rows: small-packet DMAs run at ~5 GB/s and poison
    # the 8-semaphore DMA rotation; the pad keeps packets wide while the
    # total stays small enough for the cold early-DMA window
    b1 = nc.dram_tensor("b1", [P, 512], f32, kind="ExternalInput")
    y_d = [nc.dram_tensor(f"y{j}", [P, MD * Cs[j]], dt,
                          kind="ExternalOutput") for j in range(E_LOC)]

    with TileContext(nc) as tc:
        with tc.tile_pool(name="consts", bufs=1) as consts, \
             tc.tile_pool(name="w1p", bufs=9) as w1p, \
             tc.tile_pool(name="w2p", bufs=9) as w2p, \
             tc.tile_pool(name="htp", bufs=KF) as htp, \
             tc.tile_pool(name="outp", bufs=1) as outp, \
             tc.tile_pool(name="psp", bufs=8, space="PSUM") as psp:

            xts = {}
            w1ts = {}
            w2ts = {}
            b1_t = [None]
            cur = [0]

            def emit_until(tnow):
                while cur[0] < len(events) and events[cur[0]][0] <= tnow:
                    _, ring, kind, e, i = events[cur[0]]
                    cur[0] += 1
                    eng = (nc.sync, nc.scalar, nc.gpsimd)[ring]
                    if kind == 'w1':
                        cs, w = _w1_chunks(e)[i]
                        t_ = w1p.tile([P, KD * 512], dt, tag="w1", name="w1t")
                        eng.dma_start(
                            t_[:, :KD * w],
                            w1[e, :, KD * cs:KD * (cs + w)])
                        w1ts[(e, i)] = t_
                    elif kind == 'w2':
                        t_ = w2p.tile([P, G * D_MODEL], dt, tag="w2",
                                      name="w2t")
                        eng.dma_start(t_[:], w2[e, i, :, :])
                        w2ts[(e, i)] = t_
                    elif kind == 'xt':
                        t_ = consts.tile([P, KD * Cs[e]], dt, tag=f"xt{e}",
                                         name=f"xt{e}")
                        eng.dma_start(t_[:], xt_d[e][:, :])
                        xts[e] = t_
                    else:
                        t_ = consts.tile([P, 512], f32, tag="b1")
                        eng.dma_start(t_[:], b1[:, :])
                        b1_t[0] = t_

            # startup loads (all deadlines <= 0) + PE warmup (keeps the PE
            # busy through the startup DMA window; zeroed scratch, results
            # never read)
            emit_until(0.0)
            wsc = consts.tile([P, 384], dt, tag="wsc")
            nc.vector.memset(wsc[:], 0.0)
            # dummy gelu: forces the lazy ACT gelu-table load to happen now
            # (in the startup shadow) instead of before the first real gelu
            gsc = consts.tile([P, 8], dt, tag="gsc")
            nc.scalar.activation(gsc[:], wsc[:, :8],
                                 mybir.ActivationFunctionType.Gelu)
            wps = psp.tile([P, 512], f32, tag="ps")
            for _ in range(NWARM):
                nc.tensor.matmul(wps[:, :384], wsc[:, :P], wsc[:],
                                 start=True, stop=True)

            mt = 0.0
            for _ in range(repeat):
                for e in range(E_LOC):
                    C = Cs[e]
                    chunks = _w1_chunks(e)
                    # phase A: ht[m] = gelu(w1.T @ x + b1)  [P dff x C tok]
                    hts = []
                    for m in range(KF):
                        emit_until(mt)
                        pa = psp.tile([P, 512], f32, tag="ps", name="pa")
                        col = m * P
                        ci = next(i for i, (cs, w) in enumerate(chunks)
                                  if cs <= col < cs + w)
                        cs, w = chunks[ci]
                        lc = (col - cs) // P
                        for k in range(KD):
                            nc.tensor.matmul(
                                pa[:, :C],
                                w1ts[(e, ci)][:, k * w + lc * P:
                                              k * w + (lc + 1) * P],
                                xts[e][:, k * C:k * C + C],
                                start=(k == 0), stop=(k == KD - 1))
                        ht = htp.tile([P, C0], dt, tag="ht", name="ht")
                        nc.scalar.activation(
                            ht[:, :C], pa[:, :C],
                            mybir.ActivationFunctionType.Gelu,
                            bias=b1_t[0][:, e * KF + m: e * KF + m + 1])
                        hts.append(ht)
                        mt += tA[e] / KF
                    # phase B (transposed): pb[md] = sum_k w2_k.T @ ht_k,
                    # swept as a diagonal wavefront (bank md starts at step
                    # md) so bank md's first MM comes ~md*C cycles into B —
                    # the trailing gelus (which free the PSUM banks) get
                    # ~5 us of slack instead of ~1.4, killing the A->B stall.
                    # Bank md likewise finishes at step 31+md, so the DVE
                    # drains stagger naturally; y ships as wide DMAs.
                    pbs = [psp.tile([P, 512], f32, tag="ps", name="pb")
                           for _md in range(MD)]
                    ot = outp.tile([P, MD * C0], dt, tag="out", name="ot")
                    for s in range(KF + MD - 1):
                        emit_until(mt)
                        n_mm = 0
                        for md in range(MD):
                            k = s - md
                            if not 0 <= k < KF:
                                continue
                            n_mm += 1
                            kc, g = divmod(k, G)
                            nc.tensor.matmul(
                                pbs[md][:, :C],
                                w2ts[(e, kc)][:, g * D_MODEL + md * P:
                                              g * D_MODEL + (md + 1) * P],
                                hts[k][:, :C],
                                start=(k == 0), stop=(k == KF - 1))
                        if s >= KF - 1:
                            md = s - (KF - 1)
                            nc.vector.tensor_scalar_mul(
                                ot[:, md * C:(md + 1) * C],
                                pbs[md][:, :C], 1.0)
                            if md == 3:
                                nc.sync.dma_start(y_d[e][:, :4 * C],
                                                  ot[:, :4 * C])
                            elif md == 5 and e == E_LOC - 1:
                                # last expert: ship the tail in two parallel
                                # halves on both HWDGE queues (scalar idle)
                                nc.scalar.dma_start(y_d[e][:, 4 * C:6 * C],
                                                    ot[:, 4 * C:6 * C])
                            elif md == MD - 1:
                                lo = 6 * C if e == E_LOC - 1 else 4 * C
                                nc.sync.dma_start(y_d[e][:, lo:MD * C],
                                                  ot[:, lo:MD * C])
                        mt += n_mm * tA[e] / 256.0
            # trailing dummy matmuls: keep the PE busy (HAM clock at K=8/8)
            # while the last drains + y DMA + end-of-NEFF barrier run; sized
            # to finish before the final y transfer completes
            wpe = psp.tile([P, 512], f32, tag="ps", name="wpe")
            for _ in range(16):
                nc.tensor.matmul(wpe[:, :384], wsc[:, :P], wsc[:],
                                 start=True, stop=True)
    nc.finalize()
    return nc


def _route(x, gate_w, gate_b):
    """Top-2 routing on host. Returns flattened (expert, prob) per routed
    pair, the by-expert sort order, per-expert counts/starts, and each
    pair's position within its expert segment."""
    T = x.shape[0]
    scores = x @ gate_w + gate_b                      # [T, E]
    part = np.argpartition(scores, -TOP_K, axis=1)[:, -TOP_K:]   # [T, 2]
    vals = np.take_along_axis(scores, part, axis=1)
    vmax = vals.max(axis=1, keepdims=True)
    ex = np.exp(vals - vmax)
    prob = ex / ex.sum(axis=1, keepdims=True)

    expert_flat = part.ravel()                        # [2T]
    prob_flat = prob.ravel().astype(np.float32)
    token_flat = np.repeat(np.arange(T), TOP_K)

    order = np.argsort(expert_flat, kind="stable")
    counts = np.bincount(expert_flat, minlength=NUM_EXPERTS)
    starts = np.zeros(NUM_EXPERTS + 1, dtype=np.int64)
    np.cumsum(counts, out=starts[1:])

    inv_order = np.empty_like(order)
    inv_order[order] = np.arange(order.size)
    pos = inv_order - starts[expert_flat]
    return (expert_flat, prob_flat, token_flat, order, counts, starts, pos)


def _prepare(x, gate_w, gate_b, w1, b1, w2, b2):
    """Host-side routing, balanced expert->(core,slot) assignment, and
    per-core input packing. Returns (in_maps, Cs, meta-for-combine)."""
    B, S, D = x.shape
    T = B * S
    xf = np.ascontiguousarray(x.reshape(T, D), dtype=np.float32)

    (expert_flat, prob_flat, token_flat, order, counts, starts, pos) = _route(
        xf, np.asarray(gate_w, np.float32), np.asarray(gate_b, np.float32))

    # balanced assignment: slot j of core c holds expert_desc[j*8 + c]
    expert_desc = np.argsort(-counts, kind="stable")
    core_of = np.empty(NUM_EXPERTS, dtype=np.int64)
    slot_of = np.empty(NUM_EXPERTS, dtype=np.int64)
    for j in range(E_LOC):
        for c in range(N_CORES):
            e = expert_desc[j * N_CORES + c]
            core_of[e] = c
            slot_of[e] = j
    Cs = []
    for j in range(E_LOC):
        mx = int(counts[expert_desc[j * N_CORES:(j + 1) * N_CORES]].max())
        Cs.append(max(16, -(-mx // 8) * 8))          # 8-aligned exact cap
        assert Cs[j] <= 512
    CT = sum(Cs)
    offs = [sum(Cs[:j]) for j in range(E_LOC)]

    xg16 = xf[token_flat[order]].astype(BF16)         # [2T, D] sorted by expert
    sorted_probs = prob_flat[order]

    w1_16 = np.asarray(w1, np.float32).astype(BF16)   # [E, D, F]
    w2_16 = np.asarray(w2, np.float32).astype(BF16)   # [E, F, D]
    b1_f = np.asarray(b1, np.float32)                 # [E, F]

    in_maps = []
    for c in range(N_CORES):
        m = {}
        w1_core = np.empty((E_LOC, P, KD * D_FF), dtype=BF16)
        w2_core = np.empty((E_LOC, KF // G, P, G * D_MODEL), dtype=BF16)
        b1_core = np.empty((E_LOC, D_FF), dtype=np.float32)
        for j in range(E_LOC):
            e = expert_desc[j * N_CORES + c]
            c_e = int(counts[e])
            xt_j = np.zeros((D, Cs[j]), dtype=BF16)
            if c_e:
                seg = slice(starts[e], starts[e] + c_e)
                xt_j[:, :c_e] = xg16[seg].T
            # xt packed [P, KD*C]: [p, k*C+c] = x[d=k*128+p, tok c]
            m[f"xt{j}"] = np.ascontiguousarray(
                xt_j.reshape(KD, P, Cs[j]).transpose(1, 0, 2)
                .reshape(P, KD * Cs[j]))
            # w1 packed per chunk: [p, (chunk-major) k, col] blocks
            w1r = w1_16[e].reshape(KD, P, D_FF)
            blocks = [w1r[:, :, cs:cs + w].transpose(1, 0, 2)
                      .reshape(P, KD * w) for cs, w in _w1_chunks(j)]
            w1_core[j] = np.concatenate(blocks, axis=1)
            # w2 packed per G-chunk: [p, g*D + d] = w2[f=(kc*G+g)*128+p, d]
            w2r = w2_16[e].reshape(KF // G, G, P, D_MODEL)
            w2_core[j] = w2r.transpose(0, 2, 1, 3).reshape(
                KF // G, P, G * D_MODEL)
            b1_core[j] = b1_f[e]
        m["w1"] = np.ascontiguousarray(w1_core)
        m["w2"] = np.ascontiguousarray(w2_core)
        b1_pack = np.zeros((P, 512), dtype=np.float32)
        b1_pack[:, :E_LOC * KF] = (
            b1_core.reshape(E_LOC, KF, P).transpose(2, 0, 1)
            .reshape(P, E_LOC * KF))
        m["b1"] = b1_pack
        in_maps.append(m)

    prv = np.zeros((CT * N_CORES,), dtype=np.float32)
    for c in range(N_CORES):
        for j in range(E_LOC):
            e = expert_desc[j * N_CORES + c]
            c_e = int(counts[e])
            if c_e:
                seg = slice(starts[e], starts[e] + c_e)
                base = c * CT + offs[j]
                prv[base:base + c_e] = sorted_probs[seg]
    meta = dict(T=T, shape=x.shape, CT=CT, offs=offs, prv=prv, Cs=Cs,
                core_of=core_of, slot_of=slot_of,
                expert_flat=expert_flat, prob_flat=prob_flat,
                token_flat=token_flat, pos=pos, b2=np.asarray(b2, np.float32))
    return in_maps, Cs, meta


def _combine(y_per_core, meta):
    """out[t] = sum of the token's two routed expert outputs (+ b2 term).
    Each per-core entry is [y0, y1, y2] with yj = [128, 8*C_j] packed as
    [p, md*C_j + c] = yT[d = md*128 + p, tok c]."""
    T = meta["T"]
    CT = meta["CT"]
    Cs = meta["Cs"]
    offs = np.asarray(meta["offs"], dtype=np.int64)
    expert_flat = meta["expert_flat"]
    cols_all = []
    for ys in y_per_core:
        for j, yj in enumerate(ys):
            cols_all.append(np.asarray(yj).reshape(P, MD, Cs[j])
                            .transpose(1, 0, 2).reshape(D_MODEL, Cs[j]))
    yt = np.concatenate(cols_all, axis=1).astype(np.float32)  # [D, 8*CT]

    cols = (meta["core_of"][expert_flat] * CT
            + offs[meta["slot_of"][expert_flat]] + meta["pos"])
    cols = cols.reshape(T, TOP_K)
    prv = meta["prv"]
    out = (yt[:, cols[:, 0]] * prv[cols[:, 0]]
           + yt[:, cols[:, 1]] * prv[cols[:, 1]]).T    # [T, D]

    b2_f = meta["b2"]
    if np.any(b2_f):
        combine = np.zeros((T, NUM_EXPERTS), dtype=np.float32)
        np.add.at(combine, (meta["token_flat"], expert_flat), meta["prob_flat"])
        out = out + combine @ b2_f
    return np.ascontiguousarray(out.reshape(meta["shape"]), dtype=np.float32)


def kernel(x, gate_w, gate_b, w1, b1, w2, b2):
    from concourse import bass_utils

    in_maps, Cs, meta = _prepare(x, gate_w, gate_b, w1, b1, w2, b2)
    nc = _build(Cs)
    res = bass_utils.run_bass_kernel_spmd(nc, in_maps, core_ids=list(range(N_CORES)))
    return _combine([[res.results[c][f"y{j}"] for j in range(E_LOC)]
                     for c in range(N_CORES)], meta)
